# revision 14
# baseline (speedup 1.0000x reference)
"""Bahdanau additive attention on 8 Trainium2 NeuronCores.

Shapes (hardcoded from the problem spec):
  encoder_out [B=4, Te=512, De=512], decoder_out [B=4, Td=256, Dd=512]
  W1 [512,128], W2 [512,128], V [128,1]; U=128.
Outputs: context [4,256,512], attn_weights [4,256,512].

Sharding: core c handles batch b=c//2, decoder rows (c%2)*128..+128.

Per-core pipeline (U=128 on SBUF partitions for phase 1):
  encT    = PE-transpose(enc)                  [De,Te]   (per-te-chunk pipelined)
  enc_pT  = W1^T @ encT (+b1)                  [U,Te]    (fp32r single-pass)
  dec_pT  = W2^T @ decT (+b2)                  [U,Td]
  per td: pre = enc_pT + dec_pT[:,td]          (DVE/GPSIMD tensor_scalar_add)
          h   = tanh(pre) -> bf16              (ACT, batched over SUB tds)
  scores land in NATURAL [td, te] layout via accumulating matmuls with a
  sliding-window stationary (v in column j):
          score[32-group] += Zwin_j^T @ h_td_j   (PE, bf16 moving N=512)
  softmax row-wise: exp+accum_out (ACT) -> rinv (DVE) -> attn = esc*rinv
  ctx per 32-td group: PE-transpose attn quarter -> ctx = attnT^T @ enc (fp32r)
"""

import numpy as np

B, TE, TD, DE, U = 4, 512, 256, 512, 128
N_CORES = 8
ROWS = 128  # decoder rows per core
SUB = 16  # tds per tanh batch
GRP = 32  # tds per score accumulation group / ctx quarter

_CACHE = {}


def _build_program():
    from contextlib import ExitStack

    import concourse.bacc as bacc
    import concourse.tile as tile
    from concourse import mybir
    from concourse.masks import make_identity

    f32 = mybir.dt.float32
    f32r = mybir.dt.float32r
    bf16 = mybir.dt.bfloat16
    AF = mybir.ActivationFunctionType

    nc = bacc.Bacc("TRN2", target_bir_lowering=False, debug=False)

    enc_d = nc.dram_tensor("enc", [TE, DE], f32, kind="ExternalInput")
    dec_d = nc.dram_tensor("dec", [ROWS, DE], f32, kind="ExternalInput")
    w1_d = nc.dram_tensor("w1", [DE, U], f32, kind="ExternalInput")
    w2_d = nc.dram_tensor("w2", [DE, U], f32, kind="ExternalInput")
    v_d = nc.dram_tensor("v", [U, 1], f32, kind="ExternalInput")
    w1b_d = nc.dram_tensor("w1b", [U], f32, kind="ExternalInput")
    w2b_d = nc.dram_tensor("w2b", [U], f32, kind="ExternalInput")
    ctx_d = nc.dram_tensor("ctx", [ROWS, DE], f32, kind="ExternalOutput")
    attn_d = nc.dram_tensor("attn", [ROWS, TE], f32, kind="ExternalOutput")

    NT = TE // 128  # te chunks
    ND = DE // 128  # de chunks

    with tile.TileContext(nc) as tc, ExitStack() as ctx:
        const = ctx.enter_context(tc.tile_pool(name="const", bufs=1))
        work = ctx.enter_context(tc.tile_pool(name="work", bufs=2))
        att = ctx.enter_context(tc.tile_pool(name="att", bufs=2))
        ps_t = ctx.enter_context(tc.tile_pool(name="ps_t", bufs=2, space="PSUM"))
        ps_p = ctx.enter_context(tc.tile_pool(name="ps_p", bufs=1, space="PSUM"))
        ps_v = ctx.enter_context(tc.tile_pool(name="ps_v", bufs=2, space="PSUM"))
        ps_c = ctx.enter_context(tc.tile_pool(name="ps_c", bufs=2, space="PSUM"))

        ident = const.tile([128, 128], f32, tag="ident")
        make_identity(nc, ident)

        # --- input DMAs, split across the two HWDGE rings ---
        enc_sb = [
            const.tile([128, DE], f32, tag=f"enc_{t}", name=f"enc_{t}")
            for t in range(NT)
        ]
        dec_sb = const.tile([ROWS, DE], f32, tag="dec")
        # ring1 (SP): enc0, w2, w1, enc2 ...; ring2 (ACT): dec, enc1, enc3 ...
        nc.sync.dma_start(out=enc_sb[0], in_=enc_d[0:128, :])
        nc.scalar.dma_start(out=dec_sb, in_=dec_d[:, :])
        w2_sb = const.tile([128, ND, U], f32, tag="w2")
        nc.sync.dma_start(out=w2_sb, in_=w2_d.rearrange("(k p) u -> p k u", p=128))
        w1_sb = const.tile([128, ND, U], f32, tag="w1")
        nc.sync.dma_start(out=w1_sb, in_=w1_d.rearrange("(k p) u -> p k u", p=128))
        nc.scalar.dma_start(out=enc_sb[1], in_=enc_d[128:256, :])
        nc.sync.dma_start(out=enc_sb[2], in_=enc_d[256:384, :])
        nc.scalar.dma_start(out=enc_sb[3], in_=enc_d[384:512, :])
        v_sb = const.tile([U, 1], f32, tag="v")
        nc.sync.dma_start(out=v_sb, in_=v_d[:, :])
        w1b_sb = const.tile([U, 1], f32, tag="w1b")
        nc.sync.dma_start(out=w1b_sb, in_=w1b_d[:, None])
        w2b_sb = const.tile([U, 1], f32, tag="w2b")
        nc.scalar.dma_start(out=w2b_sb, in_=w2b_d[:, None])
        w1_r = const.tile([128, ND, U], f32r, tag="w1r")
        nc.vector.tensor_copy(w1_r, w1_sb)
        w2_r = const.tile([128, ND, U], f32r, tag="w2r")
        nc.vector.tensor_copy(w2_r, w2_sb)
        enc_r = []
        for t in range(NT):
            er = const.tile([128, DE], f32r, tag=f"encr_{t}", name=f"encr_{t}")
            nc.vector.tensor_copy(er, enc_sb[t])
            enc_r.append(er)

        # sliding-window stationary: Zwin[:, (GRP-1)-j : (2*GRP-1)-j] puts
        # v (bf16) in column j of a [U, GRP] stationary, zeros elsewhere
        zwin = const.tile([U, 2 * GRP - 1], bf16, tag="zwin")
        nc.vector.memset(zwin, 0.0)
        nc.vector.tensor_copy(zwin[:, GRP - 1 : GRP], v_sb)

        # --- setup: enc chunk0 + dec first (early tanh start), then chunks 1-3 ---
        # encT stored d-major: encT_d [de-part, te] f32r
        encT = [
            const.tile([128, TE], f32r, tag=f"encT_{d}", name=f"encT_{d}")
            for d in range(ND)
        ]
        ep = ps_p.tile([U, TE], f32, tag="ep", name="ep")
        enc_pT = const.tile([U, TE], f32, tag="enc_pT")

        def enc_chunk_transpose(t):
            tp = ps_t.tile([128, ND, 128], f32, tag="tp", name=f"tp_e{t}")
            for d in range(ND):
                nc.tensor.transpose(
                    tp[:, d, :], enc_sb[t][:, d * 128 : (d + 1) * 128], ident
                )
            for d in range(ND):
                nc.vector.tensor_copy(encT[d][:, t * 128 : (t + 1) * 128], tp[:, d, :])

        # chunk 0: transpose + proj (N=128) + bias, ASAP
        enc_chunk_transpose(0)
        for d in range(ND):
            nc.tensor.matmul(
                ep[:, 0:128],
                w1_r[:, d, :],
                encT[d][:, 0:128],
                start=(d == 0),
                stop=(d == ND - 1),
            )
        nc.vector.tensor_scalar_add(enc_pT[:, 0:128], ep[:, 0:128], w1b_sb)

        # dec: transpose + proj + bias, right after chunk 0
        tpd = ps_t.tile([128, ND, 128], f32, tag="tp", name="tp_d")
        for d in range(ND):
            nc.tensor.transpose(tpd[:, d, :], dec_sb[:, d * 128 : (d + 1) * 128], ident)
        decT = const.tile([128, ND, 128], f32r, tag="decT")
        nc.vector.tensor_copy(decT, tpd)
        dp = ps_p.tile([U, ROWS], f32, tag="dp", name="dp")
        for d in range(ND):
            nc.tensor.matmul(
                dp,
                w2_r[:, d, :],
                decT[:, d, :],
                start=(d == 0),
                stop=(d == ND - 1),
            )
        dec_pT = const.tile([U, ROWS], f32, tag="dec_pT")
        nc.vector.tensor_scalar_add(dec_pT, dp, w2b_sb)

        # chunks 1-3: transposes, then one merged N=384 projection
        for t in range(1, NT):
            enc_chunk_transpose(t)
        for d in range(ND):
            nc.tensor.matmul(
                ep[:, 128:TE],
                w1_r[:, d, :],
                encT[d][:, 128:TE],
                start=(d == 0),
                stop=(d == ND - 1),
            )
        nc.vector.tensor_scalar_add(enc_pT[:, 128:TE], ep[:, 128:TE], w1b_sb)

        # --- adds + tanh + score accumulation ---
        # score tiles: [64, 512] psum, two 32-td groups each (bases 0/32)
        attn_sb = const.tile([ROWS, TE], f32, tag="attn_sb")
        n_half = ROWS // 64
        # tanh batch schedule per half: (start row in half, count, chunked)
        ramp_sched = [(0, 8, True), (8, 8, False)] + [
            (s, SUB, False) for s in range(16, 64, SUB)
        ]
        full_sched = [(s, SUB, False) for s in range(0, 64, SUB)]
        last_sched = [(0, 16, False), (16, 16, False), (32, 16, False),
                      (48, 8, False), (56, 4, False), (60, 4, False)]
        vouts = []
        for half in range(n_half):
            vout = ps_v.tile([64, TE], f32, tag="vout", name=f"vout{half}")
            vouts.append(vout)
            sched = ramp_sched if half == 0 else last_sched
            for s0, ns, chunked in sched:
                pre = work.tile([128, SUB, TE], f32, tag="pre", name="pre")
                th = work.tile([128, SUB, TE], bf16, tag="th", name="th")
                if chunked:
                    # te-chunked: start on enc_pT chunk 0 before full proj
                    for t in range(NT):
                        sl = slice(t * 128, (t + 1) * 128)
                        for j in range(ns):
                            td = half * 64 + s0 + j
                            nc.vector.tensor_scalar_add(
                                pre[:, j, sl], enc_pT[:, sl], dec_pT[:, td : td + 1]
                            )
                        nc.scalar.activation(
                            th[:, :ns, sl], pre[:, :ns, sl], AF.Tanh
                        )
                else:
                    for j in range(ns):
                        td = half * 64 + s0 + j
                        nc.vector.tensor_scalar_add(
                            pre[:, j, :], enc_pT, dec_pT[:, td : td + 1]
                        )
                    nc.scalar.activation(th[:, :ns, :], pre[:, :ns, :], AF.Tanh)
                for j in range(ns):
                    r = s0 + j  # row within this 64-row vout tile
                    g = r // GRP  # 0 or 1
                    jj = r % GRP  # position in group -> stationary column
                    nc.tensor.matmul(
                        vout[g * GRP : (g + 1) * GRP, :],
                        zwin[:, (GRP - 1) - jj : (2 * GRP - 1) - jj],
                        th[:, j, :],
                        start=(jj == 0),
                        stop=(jj == GRP - 1),
                    )

            # softmax rows (no max subtraction: |score| <= |v|_1 ~ 9)
            esc = att.tile([64, TE], f32, tag="esc", name="esc")
            esum = work.tile([64, 1], f32, tag="esum", name="esum")
            nc.scalar.activation(esc, vout, AF.Exp, accum_out=esum)
            rinv = work.tile([64, 1], f32, tag="rinv", name="rinv")
            nc.vector.reciprocal(rinv, esum)
            nc.vector.tensor_scalar_mul(
                attn_sb[half * 64 : (half + 1) * 64, :], esc, rinv
            )
            nc.sync.dma_start(
                out=attn_d[half * 64 : (half + 1) * 64, :],
                in_=attn_sb[half * 64 : (half + 1) * 64, :],
            )

        # --- context, one 64-td half at a time (partition bases 0/64 only) ---
        for q in range(ROWS // 64):
            r0 = q * 64
            at = ps_t.tile([128, NT, 64], f32, tag="tp", name=f"at{q}")
            for t in range(NT):
                nc.tensor.transpose(
                    at[:, t, :],
                    attn_sb[r0 : r0 + 64, t * 128 : (t + 1) * 128],
                    ident[r0 : r0 + 64, r0 : r0 + 64],
                )
            attnT = att.tile([128, NT, 64], f32r, tag="attnT", name="attnT")
            nc.vector.tensor_copy(attnT, at)
            ctx_ps = ps_c.tile([64, DE], f32, tag="ctx", name="ctx_ps")
            for t in range(NT):
                nc.tensor.matmul(
                    ctx_ps,
                    attnT[:, t, :],
                    enc_r[t],
                    start=(t == 0),
                    stop=(t == NT - 1),
                )
            ctx_sb = att.tile([64, DE], f32, tag="ctx_sb", name="ctx_sb")
            nc.vector.tensor_copy(ctx_sb, ctx_ps)
            nc.sync.dma_start(out=ctx_d[r0 : r0 + 64, :], in_=ctx_sb)

    nc.compile()
    return nc


def _get_nc():
    if "nc" not in _CACHE:
        _CACHE["nc"] = _build_program()
    return _CACHE["nc"]


def _install_ntff_hook():
    """The agent image's antenv lacks axon_hooks; synthesize it so
    run_bass_kernel_spmd(trace=True) can reach the boot shim's
    ctypes-based NTFF profiler."""
    import sys
    import types

    if "antenv.axon_hooks" not in sys.modules:
        mod = types.ModuleType("antenv.axon_hooks")
        mod._hook = None
        mod.set_axon_ntff_profile_hook = lambda h: setattr(mod, "_hook", h)
        mod.get_axon_ntff_profile_hook = lambda: mod._hook
        sys.modules["antenv.axon_hooks"] = mod
        try:
            from trn_agent_boot.trn_boot import _ntff_profile_via_ctypes

            mod._hook = _ntff_profile_via_ctypes("/opt/axon/libaxon_pjrt.so")
        except Exception as e:
            print(f"ntff hook install failed: {e}")
    import concourse.bass_utils as bu

    bu.upload_artifacts = lambda tmpdir: "local://" + str(tmpdir)


def run(inputs, trace=False):
    from concourse.bass_utils import run_bass_kernel_spmd

    if trace:
        _install_ntff_hook()

    nc = _get_nc()
    enc = np.asarray(inputs["encoder_out"], dtype=np.float32)
    dec = np.asarray(inputs["decoder_out"], dtype=np.float32)
    w1 = np.ascontiguousarray(inputs["W1_w"], dtype=np.float32)
    w2 = np.ascontiguousarray(inputs["W2_w"], dtype=np.float32)
    v = np.ascontiguousarray(inputs["V_w"], dtype=np.float32)
    w1b = np.ascontiguousarray(inputs["W1_b"], dtype=np.float32)
    w2b = np.ascontiguousarray(inputs["W2_b"], dtype=np.float32)

    in_maps = []
    for c in range(N_CORES):
        b, h = c // 2, c % 2
        in_maps.append(
            {
                "enc": np.ascontiguousarray(enc[b]),
                "dec": np.ascontiguousarray(dec[b, h * ROWS : (h + 1) * ROWS]),
                "w1": w1,
                "w2": w2,
                "v": v,
                "w1b": w1b,
                "w2b": w2b,
            }
        )

    res = run_bass_kernel_spmd(nc, in_maps, list(range(N_CORES)), trace=trace)

    context = np.empty((B, TD, DE), np.float32)
    attn = np.empty((B, TD, TE), np.float32)
    for c in range(N_CORES):
        b, h = c // 2, c % 2
        context[b, h * ROWS : (h + 1) * ROWS] = res.results[c]["ctx"]
        attn[b, h * ROWS : (h + 1) * ROWS] = res.results[c]["attn"]
    return (context, attn), res


def kernel(**inputs):
    (context, attn), _ = run(inputs)
    return context, attn


# revision 16
# speedup vs baseline: 1.0524x; 1.0524x over previous
"""Bahdanau additive attention on 8 Trainium2 NeuronCores.

Shapes (hardcoded from the problem spec):
  encoder_out [B=4, Te=512, De=512], decoder_out [B=4, Td=256, Dd=512]
  W1 [512,128], W2 [512,128], V [128,1]; U=128.
Outputs: context [4,256,512], attn_weights [4,256,512].

Sharding: core c handles batch b=c//2, decoder rows (c%2)*128..+128.

Per-core pipeline (U=128 on SBUF partitions for phase 1):
  encT    = PE-transpose(enc)                  [De,Te]   (per-te-chunk pipelined)
  enc_pT  = W1^T @ encT (+b1)                  [U,Te]    (fp32r single-pass)
  dec_pT  = W2^T @ decT (+b2)                  [U,Td]
  per td: pre = enc_pT + dec_pT[:,td]          (DVE/GPSIMD tensor_scalar_add)
          h   = tanh(pre) -> bf16              (ACT, batched over SUB tds)
  scores land in NATURAL [td, te] layout via accumulating matmuls with a
  sliding-window stationary (v in column j):
          score[32-group] += Zwin_j^T @ h_td_j   (PE, bf16 moving N=512)
  softmax row-wise: exp+accum_out (ACT) -> rinv (DVE) -> attn = esc*rinv
  ctx per 32-td group: PE-transpose attn quarter -> ctx = attnT^T @ enc (fp32r)
"""

import numpy as np

B, TE, TD, DE, U = 4, 512, 256, 512, 128
N_CORES = 8
ROWS = 128  # decoder rows per core
SUB = 8  # tds per tanh batch
GRP = 32  # tds per score accumulation group / ctx quarter

_CACHE = {}


def _build_program():
    from contextlib import ExitStack

    import concourse.bacc as bacc
    import concourse.tile as tile
    from concourse import mybir
    from concourse.masks import make_identity

    f32 = mybir.dt.float32
    f32r = mybir.dt.float32r
    bf16 = mybir.dt.bfloat16
    AF = mybir.ActivationFunctionType

    nc = bacc.Bacc("TRN2", target_bir_lowering=False, debug=False)

    enc_d = nc.dram_tensor("enc", [TE, DE], f32, kind="ExternalInput")
    dec_d = nc.dram_tensor("dec", [ROWS, DE], f32, kind="ExternalInput")
    w1_d = nc.dram_tensor("w1", [DE, U], f32, kind="ExternalInput")
    w2_d = nc.dram_tensor("w2", [DE, U], f32, kind="ExternalInput")
    v_d = nc.dram_tensor("v", [U, 1], f32, kind="ExternalInput")
    w1b_d = nc.dram_tensor("w1b", [U], f32, kind="ExternalInput")
    w2b_d = nc.dram_tensor("w2b", [U], f32, kind="ExternalInput")
    ctx_d = nc.dram_tensor("ctx", [ROWS, DE], f32, kind="ExternalOutput")
    attn_d = nc.dram_tensor("attn", [ROWS, TE], f32, kind="ExternalOutput")

    NT = TE // 128  # te chunks
    ND = DE // 128  # de chunks

    with tile.TileContext(nc) as tc, ExitStack() as ctx:
        const = ctx.enter_context(tc.tile_pool(name="const", bufs=1))
        work = ctx.enter_context(tc.tile_pool(name="work", bufs=3))
        att = ctx.enter_context(tc.tile_pool(name="att", bufs=2))
        ps_t = ctx.enter_context(tc.tile_pool(name="ps_t", bufs=2, space="PSUM"))
        ps_p = ctx.enter_context(tc.tile_pool(name="ps_p", bufs=1, space="PSUM"))
        ps_v = ctx.enter_context(tc.tile_pool(name="ps_v", bufs=2, space="PSUM"))
        ps_c = ctx.enter_context(tc.tile_pool(name="ps_c", bufs=2, space="PSUM"))

        ident = const.tile([128, 128], f32, tag="ident")
        make_identity(nc, ident)

        # --- input DMAs, split across the two HWDGE rings ---
        enc_sb = [
            const.tile([128, DE], f32, tag=f"enc_{t}", name=f"enc_{t}")
            for t in range(NT)
        ]
        dec_sb = const.tile([ROWS, DE], f32, tag="dec")
        # ring1 (SP): enc0, w2, w1, enc2 ...; ring2 (ACT): dec, enc1, enc3 ...
        nc.sync.dma_start(out=enc_sb[0], in_=enc_d[0:128, :])
        nc.scalar.dma_start(out=dec_sb, in_=dec_d[:, :])
        w2_sb = const.tile([128, ND, U], f32, tag="w2")
        nc.sync.dma_start(out=w2_sb, in_=w2_d.rearrange("(k p) u -> p k u", p=128))
        w1_sb = const.tile([128, ND, U], f32, tag="w1")
        nc.sync.dma_start(out=w1_sb, in_=w1_d.rearrange("(k p) u -> p k u", p=128))
        nc.scalar.dma_start(out=enc_sb[1], in_=enc_d[128:256, :])
        nc.sync.dma_start(out=enc_sb[2], in_=enc_d[256:384, :])
        nc.scalar.dma_start(out=enc_sb[3], in_=enc_d[384:512, :])
        v_sb = const.tile([U, 1], f32, tag="v")
        nc.sync.dma_start(out=v_sb, in_=v_d[:, :])
        w1b_sb = const.tile([U, 1], f32, tag="w1b")
        nc.sync.dma_start(out=w1b_sb, in_=w1b_d[:, None])
        w2b_sb = const.tile([U, 1], f32, tag="w2b")
        nc.scalar.dma_start(out=w2b_sb, in_=w2b_d[:, None])
        w1_r = const.tile([128, ND, U], f32r, tag="w1r")
        nc.vector.tensor_copy(w1_r, w1_sb)
        w2_r = const.tile([128, ND, U], f32r, tag="w2r")
        nc.vector.tensor_copy(w2_r, w2_sb)
        enc_r = []
        for t in range(NT):
            er = const.tile([128, DE], f32r, tag=f"encr_{t}", name=f"encr_{t}")
            nc.vector.tensor_copy(er, enc_sb[t])
            enc_r.append(er)

        # sliding-window stationary: Zwin[:, (GRP-1)-j : (2*GRP-1)-j] puts
        # v (bf16) in column j of a [U, GRP] stationary, zeros elsewhere
        zwin = const.tile([U, 2 * GRP - 1], bf16, tag="zwin")
        nc.vector.memset(zwin, 0.0)
        nc.vector.tensor_copy(zwin[:, GRP - 1 : GRP], v_sb)

        # --- setup: enc chunk0 + dec first (early tanh start), then chunks 1-3 ---
        # encT stored d-major: encT_d [de-part, te] f32r
        encT = [
            const.tile([128, TE], f32r, tag=f"encT_{d}", name=f"encT_{d}")
            for d in range(ND)
        ]
        ep = ps_p.tile([U, TE], f32, tag="ep", name="ep")
        enc_pT = const.tile([U, TE], bf16, tag="enc_pT")

        def enc_chunk_transpose(t):
            tp = ps_t.tile([128, ND, 128], f32, tag="tp", name=f"tp_e{t}")
            for d in range(ND):
                nc.tensor.transpose(
                    tp[:, d, :], enc_sb[t][:, d * 128 : (d + 1) * 128], ident
                )
            for d in range(ND):
                nc.vector.tensor_copy(encT[d][:, t * 128 : (t + 1) * 128], tp[:, d, :])

        # chunk 0: transpose + proj (N=128) + bias, ASAP
        enc_chunk_transpose(0)
        for d in range(ND):
            nc.tensor.matmul(
                ep[:, 0:128],
                w1_r[:, d, :],
                encT[d][:, 0:128],
                start=(d == 0),
                stop=(d == ND - 1),
            )
        nc.vector.tensor_scalar_add(enc_pT[:, 0:128], ep[:, 0:128], w1b_sb)

        # dec: transpose + proj + bias, right after chunk 0
        tpd = ps_t.tile([128, ND, 128], f32, tag="tp", name="tp_d")
        for d in range(ND):
            nc.tensor.transpose(tpd[:, d, :], dec_sb[:, d * 128 : (d + 1) * 128], ident)
        decT = const.tile([128, ND, 128], f32r, tag="decT")
        nc.vector.tensor_copy(decT, tpd)
        dp = ps_p.tile([U, ROWS], f32, tag="dp", name="dp")
        for d in range(ND):
            nc.tensor.matmul(
                dp,
                w2_r[:, d, :],
                decT[:, d, :],
                start=(d == 0),
                stop=(d == ND - 1),
            )
        dec_pT = const.tile([U, ROWS], f32, tag="dec_pT")
        nc.vector.tensor_scalar_add(dec_pT, dp, w2b_sb)

        # chunks 1-3: transposes, then one merged N=384 projection
        for t in range(1, NT):
            enc_chunk_transpose(t)
        for d in range(ND):
            nc.tensor.matmul(
                ep[:, 128:TE],
                w1_r[:, d, :],
                encT[d][:, 128:TE],
                start=(d == 0),
                stop=(d == ND - 1),
            )
        nc.vector.tensor_scalar_add(enc_pT[:, 128:TE], ep[:, 128:TE], w1b_sb)

        # --- adds + tanh + score accumulation ---
        # score tiles: [64, 512] psum, two 32-td groups each (bases 0/32)
        attn_sb = const.tile([ROWS, TE], f32, tag="attn_sb")
        n_half = ROWS // 64
        # tanh batch schedule per half: (start row in half, count, chunked)
        ramp_sched = [(0, 8, True)] + [(s, SUB, False) for s in range(8, 64, SUB)]
        full_sched = [(s, SUB, False) for s in range(0, 64, SUB)]
        last_sched = [(s, SUB, False) for s in range(0, 48, SUB)] + [
            (48, 8, False), (56, 4, False), (60, 4, False)]
        vouts = []
        for half in range(n_half):
            vout = ps_v.tile([64, TE], f32, tag="vout", name=f"vout{half}")
            vouts.append(vout)
            sched = ramp_sched if half == 0 else last_sched
            for s0, ns, chunked in sched:
                pre = work.tile([128, SUB, TE], bf16, tag="pre", name="pre")
                th = work.tile([128, SUB, TE], bf16, tag="th", name="th")
                if chunked:
                    # te-chunked: start on enc_pT chunk 0 before full proj
                    for t in range(NT):
                        sl = slice(t * 128, (t + 1) * 128)
                        for j in range(ns):
                            td = half * 64 + s0 + j
                            nc.vector.tensor_scalar_add(
                                pre[:, j, sl], enc_pT[:, sl], dec_pT[:, td : td + 1]
                            )
                        nc.scalar.activation(
                            th[:, :ns, sl], pre[:, :ns, sl], AF.Tanh
                        )
                else:
                    for j in range(ns):
                        td = half * 64 + s0 + j
                        nc.vector.tensor_scalar_add(
                            pre[:, j, :], enc_pT, dec_pT[:, td : td + 1]
                        )
                    nc.scalar.activation(th[:, :ns, :], pre[:, :ns, :], AF.Tanh)
                for j in range(ns):
                    r = s0 + j  # row within this 64-row vout tile
                    g = r // GRP  # 0 or 1
                    jj = r % GRP  # position in group -> stationary column
                    nc.tensor.matmul(
                        vout[g * GRP : (g + 1) * GRP, :],
                        zwin[:, (GRP - 1) - jj : (2 * GRP - 1) - jj],
                        th[:, j, :],
                        start=(jj == 0),
                        stop=(jj == GRP - 1),
                    )

            # softmax rows (no max subtraction: |score| <= |v|_1 ~ 9)
            esc = att.tile([64, TE], f32, tag="esc", name="esc")
            esum = work.tile([64, 1], f32, tag="esum", name="esum")
            nc.scalar.activation(esc, vout, AF.Exp, accum_out=esum)
            rinv = work.tile([64, 1], f32, tag="rinv", name="rinv")
            nc.vector.reciprocal(rinv, esum)
            nc.vector.tensor_scalar_mul(
                attn_sb[half * 64 : (half + 1) * 64, :], esc, rinv
            )
            nc.sync.dma_start(
                out=attn_d[half * 64 : (half + 1) * 64, :],
                in_=attn_sb[half * 64 : (half + 1) * 64, :],
            )

        # --- context, one 64-td half at a time (partition bases 0/64 only) ---
        for q in range(ROWS // 64):
            r0 = q * 64
            at = ps_t.tile([128, NT, 64], f32, tag="tp", name=f"at{q}")
            for t in range(NT):
                nc.tensor.transpose(
                    at[:, t, :],
                    attn_sb[r0 : r0 + 64, t * 128 : (t + 1) * 128],
                    ident[r0 : r0 + 64, r0 : r0 + 64],
                )
            attnT = att.tile([128, NT, 64], f32r, tag="attnT", name="attnT")
            nc.vector.tensor_copy(attnT, at)
            ctx_ps = ps_c.tile([64, DE], f32, tag="ctx", name="ctx_ps")
            for t in range(NT):
                nc.tensor.matmul(
                    ctx_ps,
                    attnT[:, t, :],
                    enc_r[t],
                    start=(t == 0),
                    stop=(t == NT - 1),
                )
            ctx_sb = att.tile([64, DE], f32, tag="ctx_sb", name="ctx_sb")
            nc.vector.tensor_copy(ctx_sb, ctx_ps)
            nc.sync.dma_start(out=ctx_d[r0 : r0 + 64, :], in_=ctx_sb)

    nc.compile()
    return nc


def _get_nc():
    if "nc" not in _CACHE:
        _CACHE["nc"] = _build_program()
    return _CACHE["nc"]


def _install_ntff_hook():
    """The agent image's antenv lacks axon_hooks; synthesize it so
    run_bass_kernel_spmd(trace=True) can reach the boot shim's
    ctypes-based NTFF profiler."""
    import sys
    import types

    if "antenv.axon_hooks" not in sys.modules:
        mod = types.ModuleType("antenv.axon_hooks")
        mod._hook = None
        mod.set_axon_ntff_profile_hook = lambda h: setattr(mod, "_hook", h)
        mod.get_axon_ntff_profile_hook = lambda: mod._hook
        sys.modules["antenv.axon_hooks"] = mod
        try:
            from trn_agent_boot.trn_boot import _ntff_profile_via_ctypes

            mod._hook = _ntff_profile_via_ctypes("/opt/axon/libaxon_pjrt.so")
        except Exception as e:
            print(f"ntff hook install failed: {e}")
    import concourse.bass_utils as bu

    bu.upload_artifacts = lambda tmpdir: "local://" + str(tmpdir)


def run(inputs, trace=False):
    from concourse.bass_utils import run_bass_kernel_spmd

    if trace:
        _install_ntff_hook()

    nc = _get_nc()
    enc = np.asarray(inputs["encoder_out"], dtype=np.float32)
    dec = np.asarray(inputs["decoder_out"], dtype=np.float32)
    w1 = np.ascontiguousarray(inputs["W1_w"], dtype=np.float32)
    w2 = np.ascontiguousarray(inputs["W2_w"], dtype=np.float32)
    v = np.ascontiguousarray(inputs["V_w"], dtype=np.float32)
    w1b = np.ascontiguousarray(inputs["W1_b"], dtype=np.float32)
    w2b = np.ascontiguousarray(inputs["W2_b"], dtype=np.float32)

    in_maps = []
    for c in range(N_CORES):
        b, h = c // 2, c % 2
        in_maps.append(
            {
                "enc": np.ascontiguousarray(enc[b]),
                "dec": np.ascontiguousarray(dec[b, h * ROWS : (h + 1) * ROWS]),
                "w1": w1,
                "w2": w2,
                "v": v,
                "w1b": w1b,
                "w2b": w2b,
            }
        )

    res = run_bass_kernel_spmd(nc, in_maps, list(range(N_CORES)), trace=trace)

    context = np.empty((B, TD, DE), np.float32)
    attn = np.empty((B, TD, TE), np.float32)
    for c in range(N_CORES):
        b, h = c // 2, c % 2
        context[b, h * ROWS : (h + 1) * ROWS] = res.results[c]["ctx"]
        attn[b, h * ROWS : (h + 1) * ROWS] = res.results[c]["attn"]
    return (context, attn), res


def kernel(**inputs):
    (context, attn), _ = run(inputs)
    return context, attn


# revision 17
# speedup vs baseline: 1.0616x; 1.0088x over previous
"""Bahdanau additive attention on 8 Trainium2 NeuronCores.

Shapes (hardcoded from the problem spec):
  encoder_out [B=4, Te=512, De=512], decoder_out [B=4, Td=256, Dd=512]
  W1 [512,128], W2 [512,128], V [128,1]; U=128.
Outputs: context [4,256,512], attn_weights [4,256,512].

Sharding: core c handles batch b=c//2, decoder rows (c%2)*128..+128.

Per-core pipeline (U=128 on SBUF partitions for phase 1):
  encT    = PE-transpose(enc)                  [De,Te]   (per-te-chunk pipelined)
  enc_pT  = W1^T @ encT (+b1)                  [U,Te]    (fp32r single-pass)
  dec_pT  = W2^T @ decT (+b2)                  [U,Td]
  per td: pre = enc_pT + dec_pT[:,td]          (DVE/GPSIMD tensor_scalar_add)
          h   = tanh(pre) -> bf16              (ACT, batched over SUB tds)
  scores land in NATURAL [td, te] layout via accumulating matmuls with a
  sliding-window stationary (v in column j):
          score[32-group] += Zwin_j^T @ h_td_j   (PE, bf16 moving N=512)
  softmax row-wise: exp+accum_out (ACT) -> rinv (DVE) -> attn = esc*rinv
  ctx per 32-td group: PE-transpose attn quarter -> ctx = attnT^T @ enc (fp32r)
"""

import numpy as np

B, TE, TD, DE, U = 4, 512, 256, 512, 128
N_CORES = 8
ROWS = 128  # decoder rows per core
SUB = 8  # tds per tanh batch
GRP = 32  # tds per score accumulation group / ctx quarter

_CACHE = {}


def _build_program():
    from contextlib import ExitStack

    import concourse.bacc as bacc
    import concourse.tile as tile
    from concourse import mybir
    from concourse.masks import make_identity

    f32 = mybir.dt.float32
    f32r = mybir.dt.float32r
    bf16 = mybir.dt.bfloat16
    AF = mybir.ActivationFunctionType

    nc = bacc.Bacc("TRN2", target_bir_lowering=False, debug=False)

    enc_d = nc.dram_tensor("enc", [TE, DE], f32, kind="ExternalInput")
    dec_d = nc.dram_tensor("dec", [ROWS, DE], f32, kind="ExternalInput")
    w1_d = nc.dram_tensor("w1", [DE, U], f32, kind="ExternalInput")
    w2_d = nc.dram_tensor("w2", [DE, U], f32, kind="ExternalInput")
    v_d = nc.dram_tensor("v", [U, 1], f32, kind="ExternalInput")
    w1b_d = nc.dram_tensor("w1b", [U], f32, kind="ExternalInput")
    w2b_d = nc.dram_tensor("w2b", [U], f32, kind="ExternalInput")
    ctx_d = nc.dram_tensor("ctx", [ROWS, DE], f32, kind="ExternalOutput")
    attn_d = nc.dram_tensor("attn", [ROWS, TE], f32, kind="ExternalOutput")

    NT = TE // 128  # te chunks
    ND = DE // 128  # de chunks

    with tile.TileContext(nc) as tc, ExitStack() as ctx:
        const = ctx.enter_context(tc.tile_pool(name="const", bufs=1))
        work = ctx.enter_context(tc.tile_pool(name="work", bufs=3))
        att = ctx.enter_context(tc.tile_pool(name="att", bufs=2))
        ps_t = ctx.enter_context(tc.tile_pool(name="ps_t", bufs=2, space="PSUM"))
        ps_p = ctx.enter_context(tc.tile_pool(name="ps_p", bufs=1, space="PSUM"))
        ps_v = ctx.enter_context(tc.tile_pool(name="ps_v", bufs=2, space="PSUM"))
        ps_c = ctx.enter_context(tc.tile_pool(name="ps_c", bufs=2, space="PSUM"))

        ident = const.tile([128, 128], f32, tag="ident")
        make_identity(nc, ident)

        # --- input DMAs, split across the two HWDGE rings ---
        enc_sb = [
            const.tile([128, DE], f32, tag=f"enc_{t}", name=f"enc_{t}")
            for t in range(NT)
        ]
        dec_sb = const.tile([ROWS, DE], f32, tag="dec")
        # ring1 (SP): enc0, w2, w1, enc2 ...; ring2 (ACT): dec, enc1, enc3 ...
        nc.sync.dma_start(out=enc_sb[0], in_=enc_d[0:128, :])
        nc.scalar.dma_start(out=dec_sb, in_=dec_d[:, :])
        w2_sb = const.tile([128, ND, U], f32, tag="w2")
        nc.sync.dma_start(out=w2_sb, in_=w2_d.rearrange("(k p) u -> p k u", p=128))
        w1_sb = const.tile([128, ND, U], f32, tag="w1")
        nc.sync.dma_start(out=w1_sb, in_=w1_d.rearrange("(k p) u -> p k u", p=128))
        nc.scalar.dma_start(out=enc_sb[1], in_=enc_d[128:256, :])
        nc.sync.dma_start(out=enc_sb[2], in_=enc_d[256:384, :])
        nc.scalar.dma_start(out=enc_sb[3], in_=enc_d[384:512, :])
        v_sb = const.tile([U, 1], f32, tag="v")
        nc.sync.dma_start(out=v_sb, in_=v_d[:, :])
        w1b_sb = const.tile([U, 1], f32, tag="w1b")
        nc.sync.dma_start(out=w1b_sb, in_=w1b_d[:, None])
        w2b_sb = const.tile([U, 1], f32, tag="w2b")
        nc.scalar.dma_start(out=w2b_sb, in_=w2b_d[:, None])
        w1_r = const.tile([128, ND, U], f32r, tag="w1r")
        nc.vector.tensor_copy(w1_r, w1_sb)
        w2_r = const.tile([128, ND, U], f32r, tag="w2r")
        nc.vector.tensor_copy(w2_r, w2_sb)
        enc_r = []
        for t in range(NT):
            er = const.tile([128, DE], f32r, tag=f"encr_{t}", name=f"encr_{t}")
            nc.vector.tensor_copy(er, enc_sb[t])
            enc_r.append(er)

        # sliding-window stationary: Zwin[:, (GRP-1)-j : (2*GRP-1)-j] puts
        # v (bf16) in column j of a [U, GRP] stationary, zeros elsewhere
        zwin = const.tile([U, 2 * GRP - 1], bf16, tag="zwin")
        nc.vector.memset(zwin, 0.0)
        nc.vector.tensor_copy(zwin[:, GRP - 1 : GRP], v_sb)

        # --- setup interleaved with early (te-chunked) tanh for tds 0..7 ---
        # encT stored d-major: encT_d [de-part, te] f32r
        encT = [
            const.tile([128, TE], f32r, tag=f"encT_{d}", name=f"encT_{d}")
            for d in range(ND)
        ]
        ep = ps_p.tile([U, TE], f32, tag="ep", name="ep")
        enc_pT = const.tile([U, TE], bf16, tag="enc_pT")
        attn_sb = const.tile([ROWS, TE], f32, tag="attn_sb")
        vout0 = ps_v.tile([64, TE], f32, tag="vout", name="vout0")
        pre_r = work.tile([128, 8, TE], bf16, tag="pre_r", bufs=1)
        th_r = work.tile([128, 8, TE], bf16, tag="th_r", bufs=1)

        dec_pT = None

        def enc_chunk(t):
            tp = ps_t.tile([128, ND, 128], f32, tag="tp", name=f"tp_e{t}")
            for d in range(ND):
                nc.tensor.transpose(
                    tp[:, d, :], enc_sb[t][:, d * 128 : (d + 1) * 128], ident
                )
            for d in range(ND):
                nc.vector.tensor_copy(encT[d][:, t * 128 : (t + 1) * 128], tp[:, d, :])
            sl = slice(t * 128, (t + 1) * 128)
            for d in range(ND):
                nc.tensor.matmul(
                    ep[:, sl],
                    w1_r[:, d, :],
                    encT[d][:, sl],
                    start=(d == 0),
                    stop=(d == ND - 1),
                )
            nc.vector.tensor_scalar_add(enc_pT[:, sl], ep[:, sl], w1b_sb)

        def ramp_chunk(t):
            sl = slice(t * 128, (t + 1) * 128)
            for j in range(8):
                nc.vector.tensor_scalar_add(
                    pre_r[:, j, sl], enc_pT[:, sl], dec_pT[:, j : j + 1]
                )
            nc.scalar.activation(th_r[:, :, sl], pre_r[:, :, sl], AF.Tanh)

        enc_chunk(0)

        # dec: transpose + proj + bias, right after chunk 0
        tpd = ps_t.tile([128, ND, 128], f32, tag="tp", name="tp_d")
        for d in range(ND):
            nc.tensor.transpose(tpd[:, d, :], dec_sb[:, d * 128 : (d + 1) * 128], ident)
        decT = const.tile([128, ND, 128], f32r, tag="decT")
        nc.vector.tensor_copy(decT, tpd)
        dp = ps_p.tile([U, ROWS], f32, tag="dp", name="dp")
        for d in range(ND):
            nc.tensor.matmul(
                dp,
                w2_r[:, d, :],
                decT[:, d, :],
                start=(d == 0),
                stop=(d == ND - 1),
            )
        dec_pT = const.tile([U, ROWS], f32, tag="dec_pT")
        nc.vector.tensor_scalar_add(dec_pT, dp, w2b_sb)

        ramp_chunk(0)
        for t in range(1, NT):
            enc_chunk(t)
            ramp_chunk(t)
        for j in range(8):
            nc.tensor.matmul(
                vout0[0:GRP, :],
                zwin[:, (GRP - 1) - j : (2 * GRP - 1) - j],
                th_r[:, j, :],
                start=(j == 0),
                stop=False,
            )

        # --- adds + tanh + score accumulation (tds 8..127) ---
        n_half = ROWS // 64
        ramp_sched = [(s, SUB, False) for s in range(8, 64, SUB)]
        last_sched = [(s, SUB, False) for s in range(0, 48, SUB)] + [
            (48, 8, False), (56, 4, False), (60, 4, False)]
        vouts = []
        for half in range(n_half):
            if half == 0:
                vout = vout0
            else:
                vout = ps_v.tile([64, TE], f32, tag="vout", name=f"vout{half}")
            vouts.append(vout)
            sched = ramp_sched if half == 0 else last_sched
            for s0, ns, chunked in sched:
                pre = work.tile([128, SUB, TE], bf16, tag="pre", name="pre")
                th = work.tile([128, SUB, TE], bf16, tag="th", name="th")
                for j in range(ns):
                    td = half * 64 + s0 + j
                    nc.vector.tensor_scalar_add(
                        pre[:, j, :], enc_pT, dec_pT[:, td : td + 1]
                    )
                nc.scalar.activation(th[:, :ns, :], pre[:, :ns, :], AF.Tanh)
                for j in range(ns):
                    r = s0 + j  # row within this 64-row vout tile
                    g = r // GRP  # 0 or 1
                    jj = r % GRP  # position in group -> stationary column
                    nc.tensor.matmul(
                        vout[g * GRP : (g + 1) * GRP, :],
                        zwin[:, (GRP - 1) - jj : (2 * GRP - 1) - jj],
                        th[:, j, :],
                        start=(jj == 0),
                        stop=(jj == GRP - 1),
                    )

            # softmax rows (no max subtraction: |score| <= |v|_1 ~ 9)
            esc = att.tile([64, TE], f32, tag="esc", name="esc")
            esum = work.tile([64, 1], f32, tag="esum", name="esum")
            nc.scalar.activation(esc, vout, AF.Exp, accum_out=esum)
            rinv = work.tile([64, 1], f32, tag="rinv", name="rinv")
            nc.vector.reciprocal(rinv, esum)
            nc.vector.tensor_scalar_mul(
                attn_sb[half * 64 : (half + 1) * 64, :], esc, rinv
            )
            nc.sync.dma_start(
                out=attn_d[half * 64 : (half + 1) * 64, :],
                in_=attn_sb[half * 64 : (half + 1) * 64, :],
            )

        # --- context, one 64-td half at a time (partition bases 0/64 only) ---
        for q in range(ROWS // 64):
            r0 = q * 64
            at = ps_t.tile([128, NT, 64], f32, tag="tp", name=f"at{q}")
            for t in range(NT):
                nc.tensor.transpose(
                    at[:, t, :],
                    attn_sb[r0 : r0 + 64, t * 128 : (t + 1) * 128],
                    ident[r0 : r0 + 64, r0 : r0 + 64],
                )
            attnT = att.tile([128, NT, 64], f32r, tag="attnT", name="attnT")
            nc.vector.tensor_copy(attnT, at)
            ctx_ps = ps_c.tile([64, DE], f32, tag="ctx", name="ctx_ps")
            for t in range(NT):
                nc.tensor.matmul(
                    ctx_ps,
                    attnT[:, t, :],
                    enc_r[t],
                    start=(t == 0),
                    stop=(t == NT - 1),
                )
            ctx_sb = att.tile([64, DE], f32, tag="ctx_sb", name="ctx_sb")
            nc.vector.tensor_copy(ctx_sb, ctx_ps)
            nc.sync.dma_start(out=ctx_d[r0 : r0 + 64, :], in_=ctx_sb)

    nc.compile()
    return nc


def _get_nc():
    if "nc" not in _CACHE:
        _CACHE["nc"] = _build_program()
    return _CACHE["nc"]


def _install_ntff_hook():
    """The agent image's antenv lacks axon_hooks; synthesize it so
    run_bass_kernel_spmd(trace=True) can reach the boot shim's
    ctypes-based NTFF profiler."""
    import sys
    import types

    if "antenv.axon_hooks" not in sys.modules:
        mod = types.ModuleType("antenv.axon_hooks")
        mod._hook = None
        mod.set_axon_ntff_profile_hook = lambda h: setattr(mod, "_hook", h)
        mod.get_axon_ntff_profile_hook = lambda: mod._hook
        sys.modules["antenv.axon_hooks"] = mod
        try:
            from trn_agent_boot.trn_boot import _ntff_profile_via_ctypes

            mod._hook = _ntff_profile_via_ctypes("/opt/axon/libaxon_pjrt.so")
        except Exception as e:
            print(f"ntff hook install failed: {e}")
    import concourse.bass_utils as bu

    bu.upload_artifacts = lambda tmpdir: "local://" + str(tmpdir)


def run(inputs, trace=False):
    from concourse.bass_utils import run_bass_kernel_spmd

    if trace:
        _install_ntff_hook()

    nc = _get_nc()
    enc = np.asarray(inputs["encoder_out"], dtype=np.float32)
    dec = np.asarray(inputs["decoder_out"], dtype=np.float32)
    w1 = np.ascontiguousarray(inputs["W1_w"], dtype=np.float32)
    w2 = np.ascontiguousarray(inputs["W2_w"], dtype=np.float32)
    v = np.ascontiguousarray(inputs["V_w"], dtype=np.float32)
    w1b = np.ascontiguousarray(inputs["W1_b"], dtype=np.float32)
    w2b = np.ascontiguousarray(inputs["W2_b"], dtype=np.float32)

    in_maps = []
    for c in range(N_CORES):
        b, h = c // 2, c % 2
        in_maps.append(
            {
                "enc": np.ascontiguousarray(enc[b]),
                "dec": np.ascontiguousarray(dec[b, h * ROWS : (h + 1) * ROWS]),
                "w1": w1,
                "w2": w2,
                "v": v,
                "w1b": w1b,
                "w2b": w2b,
            }
        )

    res = run_bass_kernel_spmd(nc, in_maps, list(range(N_CORES)), trace=trace)

    context = np.empty((B, TD, DE), np.float32)
    attn = np.empty((B, TD, TE), np.float32)
    for c in range(N_CORES):
        b, h = c // 2, c % 2
        context[b, h * ROWS : (h + 1) * ROWS] = res.results[c]["ctx"]
        attn[b, h * ROWS : (h + 1) * ROWS] = res.results[c]["attn"]
    return (context, attn), res


def kernel(**inputs):
    (context, attn), _ = run(inputs)
    return context, attn
